# revision 1
# baseline (speedup 1.0000x reference)
"""Trainium2 Bass kernel for the Dynamic MultiTeacher4 distillation loss.

Strategy (pure data parallel over the batch):
  - B=8192 rows are sharded 1024/core across 8 NeuronCores.
  - On device, each core makes ONE pass over its 5 [1024,1000] f32 tensors
    and emits 17 per-row reduction stats:
      m1..m4 : exact f32 row maxes of outputs1..4
               (tensor_scalar's TensorScalarPtrReduce form: out = in*1.0
                exactly, accum_out = max-reduce, at the 2x single-src DVE
                rate - tensor_reduce only has a 1x uop)
      mm4    : exact f32 rowmax of mimic4 = ((o1+o2)+o3)+o4.  The sum chain
               runs on the TensorEngine as 4 identity matmuls accumulating
               into PSUM - each matmul contributes exactly o_t (1.0*x + 0s)
               and PSUM accumulates f32 left-assoc, so the result is
               bit-identical to the reference's f32 add chain.
      A1..A5 : sum_c exp(t/20)  (ScalarE exp with free accum_out row-sum)
      B1..B5 : sum_c exp(t/20)*s  (bf16 DVE mult at 2x + bf16 tensor_scalar
               sum-reduce at 4x)
      S1, S2 : sum_c exp(s), sum_c exp(s/20)
    exp() needs no max-subtract: inputs are N(0,1) logits, |x| < ~7, so
    exp stays comfortably inside f32 range.
  - The first SPLIT_N row-blocks are processed as two independent C-halves
    ("virtual blocks" with their own stat slots, merged on the host by
    max/sum) so compute starts as soon as the first 256KB lands - the
    start of the kernel is DMA-paced and full blocks would idle the
    compute engines for ~7us per block.
  - Host combines the O(B) stats: margins (second-max computed lazily on
    the ~B*5/1000 rows where the target IS the row argmax), threshold
    softmax, global max_preds = max over m1..m4, CE/KD terms, final mean.

The device does all O(B*C) work; the host does O(B) work plus ~40 rows of
lazy second-max. Memory-roofline bound: 20.5 MB/core of HBM reads.
"""

import os
import time

import numpy as np

import concourse.bass as bass
import concourse.bacc as bacc
import concourse.tile as tile
from concourse import mybir
from concourse.bass_utils import run_bass_kernel_spmd
from concourse.masks import make_identity

B, C = 8192, 1000
NCORES = 8
ROWS = B // NCORES  # 1024 rows per core
P = 128
NBLK = ROWS // P  # 8 row-blocks per core
H = C // 2  # 500
SPLIT_N = int(os.environ.get("KERNEL_SPLIT_N", "0"))  # leading row-blocks split into C-halves (ramp fill)

ALPHA = 0.8
T_KD = 20.0
T_THR = 2.0

COPY_ENGINE = os.environ.get("KERNEL_COPY", "gp")  # "gp" or "dve"

_NC = None
LAST_RESULTS = None  # BassKernelResults of the most recent run (for profiling)


def _entries():
    """(row_block, c0, width, slot) for every virtual block."""
    ents = []
    slot = 0
    for i in range(NBLK):
        if i < SPLIT_N:
            ents.append((i, 0, H, slot))
            ents.append((i, H, H, slot + 1))
            slot += 2
        else:
            ents.append((i, 0, C, slot))
            slot += 1
    return ents


ENTRIES = _entries()
NSLOT = NBLK + SPLIT_N


def _build():
    f32 = mybir.dt.float32
    bf16 = mybir.dt.bfloat16
    Alu = mybir.AluOpType
    Act = mybir.ActivationFunctionType

    nc = bacc.Bacc(
        "TRN2", target_bir_lowering=False, debug=False, num_devices=NCORES
    )

    o1 = nc.dram_tensor("o1", [ROWS, C], f32, kind="ExternalInput").ap()
    o2 = nc.dram_tensor("o2", [ROWS, C], f32, kind="ExternalInput").ap()
    o3 = nc.dram_tensor("o3", [ROWS, C], f32, kind="ExternalInput").ap()
    o4 = nc.dram_tensor("o4", [ROWS, C], f32, kind="ExternalInput").ap()
    s_ = nc.dram_tensor("s", [ROWS, C], f32, kind="ExternalInput").ap()
    st_act = nc.dram_tensor("st_act", [NSLOT, P, 7], f32, kind="ExternalOutput").ap()
    st_dve = nc.dram_tensor("st_dve", [NSLOT, P, 7], f32, kind="ExternalOutput").ap()
    st_gp = nc.dram_tensor("st_gp", [NSLOT, P, 4], f32, kind="ExternalOutput").ap()

    o1r = o1.rearrange("(n p) c -> n p c", p=P)
    o2r = o2.rearrange("(n p) c -> n p c", p=P)
    o3r = o3.rearrange("(n p) c -> n p c", p=P)
    o4r = o4.rearrange("(n p) c -> n p c", p=P)
    sr = s_.rearrange("(n p) c -> n p c", p=P)
    teachers_dram = (o1r, o2r, o3r, o4r)

    with tile.TileContext(nc) as tc:
        with (
            tc.tile_pool(name="const", bufs=1) as const,
            tc.tile_pool(name="io", bufs=3) as io,
            tc.tile_pool(name="wk", bufs=3) as wk,
            tc.tile_pool(name="st", bufs=NSLOT + 1) as st,
            tc.tile_pool(name="ps", bufs=3, space="PSUM") as ps,
        ):
            ident = const.tile([P, P], f32, tag="ident")
            make_identity(nc, ident)
            # warm up the PE during the DMA-paced ramp: cold matmuls run at
            # a fraction of steady rate, and the first mimic groups sit on
            # the critical path of the first blocks' exp/dot chain
            warm = ps.tile(
                [P, 2, H], f32, tag="pm2", padded_shape=[P, 2, 512], bufs=3
            )
            for _ in range(6):
                nc.tensor.matmul(
                    warm[:, 0, 0:P], ident, ident, start=True, stop=True
                )

            stats_tiles = []
            for i, c0, w, slot in ENTRIES:
                nh = w // H
                # load order t1, s, t2, t3, t4: the student tensor lands
                # second so ACT's student exps and Pool's bf16 copy (feeding
                # every dot) start early during the DMA-paced ramp
                t = io.tile([P, w], f32, tag="t0")
                nc.sync.dma_start(out=t, in_=teachers_dram[0][i][:, c0 : c0 + w])
                tt = [t]
                ts = io.tile([P, w], f32, tag="ts")
                nc.sync.dma_start(out=ts, in_=sr[i][:, c0 : c0 + w])
                for k in (1, 2, 3):
                    t = io.tile([P, w], f32, tag=f"t{k}")
                    nc.sync.dma_start(
                        out=t, in_=teachers_dram[k][i][:, c0 : c0 + w]
                    )
                    tt.append(t)

                # -- exact f32 row maxes of the 4 teachers (2x DVE rate) --
                sg = st.tile([P, 4], f32, tag="sg")
                mscrap = wk.tile([P, w], f32, tag="mscrap")
                for k, t in enumerate(tt):
                    nc.vector.tensor_scalar(
                        out=mscrap, in0=t, scalar1=1.0, scalar2=None,
                        op0=Alu.mult, op1=Alu.max,
                        accum_out=sg[:, k : k + 1],
                    )

                # -- mimic4 = ((o1+o2)+o3)+o4 exactly on the TensorEngine,
                #    one 500-wide accumulation group per PSUM bank --
                sd = st.tile([P, 7], f32, tag="sd")
                pm = ps.tile(
                    [P, nh, H], f32, tag=f"pm{nh}",
                    padded_shape=[P, nh, 512], bufs=(3 if nh == 2 else 2),
                )
                for j in range(nh):
                    for k, t in enumerate(tt):
                        nc.tensor.matmul(
                            pm[:, j, :],
                            ident,
                            t[:, j * H : (j + 1) * H],
                            start=(k == 0),
                            stop=(k == 3),
                        )
                nc.vector.tensor_scalar(
                    out=mscrap.rearrange("p (j c) -> p j c", j=nh),
                    in0=pm, scalar1=1.0, scalar2=None,
                    op0=Alu.mult, op1=Alu.max, accum_out=sd[:, 0:1],
                )

                # -- scalar engine: 7 exp passes, accum_out row-sums free --
                sa = st.tile([P, 7], f32, tag="sa")
                es = []

                def emit_teacher_exp(k):
                    e = wk.tile([P, w], bf16, tag=f"e{k}", name=f"e{k}_{slot}")
                    nc.scalar.activation(
                        out=e, in_=tt[k], func=Act.Exp, scale=1.0 / T_KD,
                        accum_out=sa[:, k : k + 1],
                    )
                    es.append(e)

                emit_teacher_exp(0)
                def emit_em():
                    em = wk.tile([P, w], bf16, tag="em", name=f"em_{slot}")
                    nc.scalar.activation(
                        out=em.rearrange("p (j c) -> p j c", j=nh),
                        in_=pm, func=Act.Exp, scale=1.0 / (4.0 * T_KD),
                        accum_out=sa[:, 4:5],
                    )
                    es.append(em)

                last = slot == NSLOT - 1
                if last:
                    # tail: em early so the final dot chain drains sooner
                    # (PE is long done by now)
                    for k in (1, 2, 3):
                        emit_teacher_exp(k)
                    emit_em()
                scr_s = wk.tile([P, w], bf16, tag="scr_s")
                nc.scalar.activation(
                    out=scr_s, in_=ts, func=Act.Exp, scale=1.0,
                    accum_out=sa[:, 5:6],
                )
                scr_s2 = wk.tile([P, w], bf16, tag="scr_s2")
                nc.scalar.activation(
                    out=scr_s2, in_=ts, func=Act.Exp, scale=1.0 / T_KD,
                    accum_out=sa[:, 6:7],
                )
                if not last:
                    for k in (1, 2, 3):
                        emit_teacher_exp(k)
                    # em LAST on the in-order ACT queue: it depends on the
                    # PE matmul group and must not head-of-line-block
                    emit_em()

                # -- vector: B_t = sum exp(t/20)*s.  bf16 mult at 2x, then
                #    bf16 tensor_scalar sum-reduce at 4x --
                sb = wk.tile([P, w], bf16, tag="sb")
                if COPY_ENGINE == "gp":
                    nc.gpsimd.tensor_copy(out=sb, in_=ts)
                else:
                    nc.vector.tensor_copy(out=sb, in_=ts)
                scr = wk.tile([P, w], bf16, tag="scr")
                for k, e in enumerate(es):
                    prod = wk.tile([P, w], bf16, tag=f"prod{k}")
                    nc.vector.tensor_mul(out=prod, in0=e, in1=sb)
                    nc.vector.tensor_scalar(
                        out=scr, in0=prod, scalar1=1.0, scalar2=None,
                        op0=Alu.mult, op1=Alu.add,
                        accum_out=sd[:, k + 1 : k + 2],
                    )

                stats_tiles.append((slot, sa, sd, sg))

            # all stats stores after the loop: the in-order sync sequencer
            # must never block a later block's loads behind a store that
            # waits on compute. By now all loads are issued; these tiny
            # stores drain at the end.
            for slot, sa, sd, sg in stats_tiles:
                nc.sync.dma_start(out=st_act[slot], in_=sa)
                nc.sync.dma_start(out=st_dve[slot], in_=sd)
                nc.sync.dma_start(out=st_gp[slot], in_=sg)

    nc.compile()
    return nc


def _get_nc():
    global _NC
    if _NC is None:
        _NC = _build()
    return _NC


def _merge_slots(arr, op):
    """[NSLOT, P, K] per-slot stats -> [NBLK*P, K] per-row stats."""
    out = []
    for i in range(NBLK):
        slots = [s for (ib, _c0, _w, s) in ENTRIES if ib == i]
        m = arr[slots[0]]
        for s in slots[1:]:
            m = op(m, arr[s])
        out.append(m)
    return np.concatenate(out, 0)


def gather_stats(res):
    """Merge per-slot device stats into per-row [B, *] arrays."""
    sas, sds, sgs = [], [], []
    for r in res.results:
        # sums combine across C-halves by addition, maxes by max
        sas.append(_merge_slots(r["st_act"], np.add))
        sd_max = _merge_slots(r["st_dve"][:, :, 0:1], np.maximum)
        sd_sum = _merge_slots(r["st_dve"][:, :, 1:7], np.add)
        sds.append(np.concatenate([sd_max, sd_sum], 1))
        sgs.append(_merge_slots(r["st_gp"], np.maximum))
    return (
        np.concatenate(sas, 0),
        np.concatenate(sds, 0),
        np.concatenate(sgs, 0),
    )


def kernel(outputs1, outputs2, outputs3, outputs4, out_s, targets):
    global LAST_RESULTS
    # inputs may arrive as jax arrays; all downstream code (slicing, fancy
    # indexing, np.partition) assumes numpy
    outputs1 = np.asarray(outputs1, dtype=np.float32)
    outputs2 = np.asarray(outputs2, dtype=np.float32)
    outputs3 = np.asarray(outputs3, dtype=np.float32)
    outputs4 = np.asarray(outputs4, dtype=np.float32)
    out_s = np.asarray(out_s, dtype=np.float32)
    targets = np.asarray(targets)
    nc = _get_nc()

    in_maps = []
    for k in range(NCORES):
        sl = slice(k * ROWS, (k + 1) * ROWS)
        in_maps.append(
            {
                "o1": np.ascontiguousarray(outputs1[sl]),
                "o2": np.ascontiguousarray(outputs2[sl]),
                "o3": np.ascontiguousarray(outputs3[sl]),
                "o4": np.ascontiguousarray(outputs4[sl]),
                "s": np.ascontiguousarray(out_s[sl]),
            }
        )

    def _run():
        try:
            return run_bass_kernel_spmd(
                nc, in_maps, core_ids=list(range(NCORES))
            )
        except ModuleNotFoundError:
            # BASS_TRACE set but this environment lacks the axon NTFF hook
            os.environ["BASS_NEVER_TRACE"] = "1"
            return run_bass_kernel_spmd(
                nc, in_maps, core_ids=list(range(NCORES))
            )

    res = None
    for attempt in range(3):
        try:
            res = _run()
            break
        except ModuleNotFoundError:
            raise
        except Exception:
            # transient accelerator faults (NRT_EXEC_UNIT_UNRECOVERABLE) have
            # been observed on this stack lasting more than one attempt;
            # back off and retry before giving up
            if attempt == 2:
                raise
            time.sleep(15 * (attempt + 1))
    LAST_RESULTS = res

    sa, sd, sg = gather_stats(res)

    return _finalize(
        sa, sd, sg, outputs1, outputs2, outputs3, outputs4, out_s, targets
    )


def _finalize(sa, sd, sg, outputs1, outputs2, outputs3, outputs4, out_s, targets):
    f32 = np.float32
    tgt = np.asarray(targets).astype(np.int64)
    ar = np.arange(B)

    A = sa[:, 0:5].astype(np.float64)  # A1..A4, Am
    S1 = sa[:, 5].astype(np.float64)  # sum exp(s)
    S2 = sa[:, 6].astype(np.float64)  # sum exp(s/20)
    mm4 = sd[:, 0]  # rowmax of unscaled mimic4 (f32, exact)
    Bt = sd[:, 1:6].astype(np.float64)  # B1..B4, Bm
    m14 = sg  # [B,4] f32 row maxes (exact)

    # target-gathered logits (exact input f32 values)
    v1 = outputs1[ar, tgt]
    v2 = outputs2[ar, tgt]
    v3 = outputs3[ar, tgt]
    v4 = outputs4[ar, tgt]
    vs = out_s[ar, tgt]
    # mimic target value, replicating the device/reference f32 assoc exactly
    v5 = (((v1 + v2) + v3) + v4) * f32(0.25)
    m5 = mm4 * f32(0.25)  # exact rescale of the exact max

    mall = np.concatenate([m14, m5[:, None]], 1)  # [B,5] f32
    vall = np.stack([v1, v2, v3, v4, v5], 1)  # [B,5] f32

    # margins: nonzero only where the target hits the row max (~B*5/1000 rows)
    margins = np.zeros((B, 5), np.float32)
    eq_rows, eq_ts = np.nonzero(vall == mall)
    teacher_arrs = (outputs1, outputs2, outputs3, outputs4)
    for r, t in zip(eq_rows, eq_ts):
        if t < 4:
            row = teacher_arrs[t][r]
        else:
            row = (
                ((outputs1[r] + outputs2[r]) + outputs3[r]) + outputs4[r]
            ) * f32(0.25)
        m2 = np.partition(row, -2)[-2]
        margins[r, t] = mall[r, t] - m2

    z = margins.astype(np.float64) / T_THR
    ez = np.exp(z - z.max(1, keepdims=True))
    thr = ez / ez.sum(1, keepdims=True)

    max_preds = np.float64(m14.max())
    w = vall.astype(np.float64) / max_preds
    w1 = 1.0 - ALPHA * w
    w2 = ALPHA * w

    ce = np.log(S1) - vs.astype(np.float64)  # [B]
    kd = (T_KD * T_KD) * np.log(S2)[:, None] - T_KD * (Bt / A)  # [B,5]

    loss = w1 * ce[:, None] + w2 * kd
    per_sample = (thr * loss).sum(1)
    return np.asarray(per_sample.mean(), dtype=np.float32)



# revision 2
# speedup vs baseline: 1.0016x; 1.0016x over previous
"""Trainium2 Bass kernel for the Dynamic MultiTeacher4 distillation loss.

Strategy (pure data parallel over the batch):
  - B=8192 rows sharded 1024/core across 8 NeuronCores; 8 row-blocks of 128
    rows per core, processed as 4 PAIRS of blocks so every Activation pass
    covers [128, 2x1000] and the per-instruction init cost (~185ns) and
    accumulator-read cost (187ns) are amortized.
  - Teachers o1..o4 are declared float32r in DRAM (same bits as f32) so the
    mimic sum o1+o2+o3+o4 runs on the TensorEngine at 1 cycle/row instead of
    f32's 4 (the executor rounds f32r reads to ~1.5e-4 rel, absorbed by the
    host-side candidate-window recompute below). Non-PE consumers read the
    tiles through .bitcast(f32).
  - Per block, 17 per-row stats are produced:
      A1..A5 : sum_c exp(t/20)        (DVE bf16 tensor_scalar reduce, 4x rate)
      B1..B5 : sum_c exp(t/20)*s      (bf16 mul on DVE/Pool + DVE reduce)
      m1..m4, mm : max_c exp(t/20)    (DVE bf16 max-reduce; max of the exp is
                   monotone in the max of t, recovered as 20*log on host)
      S1 : sum_c exp(s)   (free accum_out on the per-block exp(s) ACT pass)
      S2 : sum_c exp(s/20) (DVE reduce over the pair-wide exp(s/20) output)
  - Engine balance per block (cost-model ns): ACT ~6.8k, DVE ~6.8k,
    Pool ~5.4k (s->bf16 copy + 2 of the 5 B-muls), PE small with f32r,
    DMA 7.1k -> everything sits just under the 57us DMA roofline/core.
  - Host combines the O(B) stats. Row maxes come back only approximately
    (bf16 exp + f32r rounding, error <= ~0.1 logits), so margin rows are
    detected by a window test v >= m~ - eps and recomputed EXACTLY on the
    host for the ~50 candidate rows (incl. exact second max); max_preds is
    recovered exactly the same way. This reproduces the reference's exact
    f32 margin semantics while the device only ships cheap approximations.
"""

import os
import time

import numpy as np

import concourse.bass as bass
import concourse.bacc as bacc
import concourse.tile as tile
from concourse import mybir
from concourse.bass_utils import run_bass_kernel_spmd
from concourse.masks import make_identity

B, C = 8192, 1000
NCORES = 8
ROWS = B // NCORES  # 1024 rows per core
P = 128
NBLK = ROWS // P  # 8 row-blocks per core
H = 500

ALPHA = 0.8
T_KD = 20.0
T_THR = 2.0

# margin-candidate windows (logit units); generous vs the ~0.08 worst-case
# bf16+f32r error so no true margin row can be missed
EPS_T = 0.15      # teachers 1..4
EPS_M = 0.60      # unscaled mimic (errors scale by T=80 on the exp side)
EPS_GLOBAL = 0.25 # max_preds recovery window

POOL_MULS = int(os.environ.get("KERNEL_POOL_MULS", "2"))  # of the 4 teacher B-muls

# stat column layout in st_dve[P, NBLK, 16]
D_A = 0      # A1..A4 at 0..3, A5 at 4
D_M = 5      # m1..m4 at 5..8, mm at 9
D_B = 10     # B1..B5 at 10..14
D_S2 = 15
ND = 16

_NC = None
LAST_RESULTS = None  # BassKernelResults of the most recent run (for profiling)

PAIRS = [(0, 1), (2, 3), (4, 5), (6, 7)]


def _build():
    f32 = mybir.dt.float32
    f32r = mybir.dt.float32r
    bf16 = mybir.dt.bfloat16
    Alu = mybir.AluOpType
    Act = mybir.ActivationFunctionType

    nc = bacc.Bacc(
        "TRN2", target_bir_lowering=False, debug=False, num_devices=NCORES
    )

    o1 = nc.dram_tensor("o1", [ROWS, C], f32r, kind="ExternalInput").ap()
    o2 = nc.dram_tensor("o2", [ROWS, C], f32r, kind="ExternalInput").ap()
    o3 = nc.dram_tensor("o3", [ROWS, C], f32r, kind="ExternalInput").ap()
    o4 = nc.dram_tensor("o4", [ROWS, C], f32r, kind="ExternalInput").ap()
    s_ = nc.dram_tensor("s", [ROWS, C], f32, kind="ExternalInput").ap()
    st_act = nc.dram_tensor("st_act", [P, NBLK], f32, kind="ExternalOutput").ap()
    st_dve = nc.dram_tensor("st_dve", [P, NBLK, ND], f32, kind="ExternalOutput").ap()

    # [p, b, c] view: element (p, b, c) = row b*128+p of the flat tensor
    t_dram = [o.rearrange("(b p) c -> p b c", p=P) for o in (o1, o2, o3, o4)]
    s_dram = s_.rearrange("(b p) c -> p b c", p=P)

    with tile.TileContext(nc) as tc:
        with (
            tc.tile_pool(name="const", bufs=1) as const,
            tc.tile_pool(name="io", bufs=2) as io,
            tc.tile_pool(name="wk", bufs=2) as wk,
            tc.tile_pool(name="st", bufs=1) as st,
            tc.tile_pool(name="ps", bufs=2, space="PSUM") as ps,
        ):
            ident_f = const.tile([P, P], f32, tag="ident_f")
            make_identity(nc, ident_f)
            ident = const.tile([P, P], f32r, tag="ident")
            nc.vector.tensor_copy(out=ident, in_=ident_f)

            sa = st.tile([P, NBLK], f32, tag="sa")
            sd = st.tile([P, NBLK, ND], f32, tag="sd")

            # PE warmup: ramps the p-state before the first real mimic sums
            warm = ps.tile([P, 4, H], f32, tag="pm", padded_shape=[P, 4, 512])
            for _ in range(6):
                nc.tensor.matmul(
                    warm[:, 0, 0:P], ident, ident, start=True, stop=True
                )

            for g, (j0, j1) in enumerate(PAIRS):
                blks = (j0, j1)

                # ---- loads: t1, s, t2, t3, t4 ----
                tt = []
                t = io.tile([P, 2, C], f32r, tag="t0")
                nc.sync.dma_start(out=t, in_=t_dram[0][:, j0 : j1 + 1, :])
                tt.append(t)
                ts = io.tile([P, 2, C], f32, tag="ts")
                nc.sync.dma_start(out=ts, in_=s_dram[:, j0 : j1 + 1, :])
                for k in (1, 2, 3):
                    t = io.tile([P, 2, C], f32r, tag=f"t{k}")
                    nc.sync.dma_start(out=t, in_=t_dram[k][:, j0 : j1 + 1, :])
                    tt.append(t)

                # ---- Pool: bf16 copy of s (feeds every B-mul) ----
                sb = wk.tile([P, 2, C], bf16, tag="sb")
                nc.gpsimd.tensor_copy(out=sb, in_=ts)

                # ---- ACT passes (pair-wide [128,2000] unless noted) ----
                ascrap = wk.tile([P, 2, C], bf16, tag="ascrap")
                es = []
                e = wk.tile([P, 2, C], bf16, tag="e0")
                nc.scalar.activation(
                    out=e, in_=tt[0].bitcast(f32), func=Act.Exp, scale=1.0 / T_KD
                )
                es.append(e)
                # per-block exp(s) with free S1 row-sum accumulators
                for jj, j in enumerate(blks):
                    nc.scalar.activation(
                        out=ascrap[:, jj, :], in_=ts[:, jj, :], func=Act.Exp,
                        scale=1.0, accum_out=sa[:, j : j + 1],
                    )
                scr2 = wk.tile([P, 2, C], bf16, tag="scr2")
                nc.scalar.activation(
                    out=scr2, in_=ts, func=Act.Exp, scale=1.0 / T_KD
                )
                for k in (1, 2, 3):
                    e = wk.tile([P, 2, C], bf16, tag=f"e{k}")
                    nc.scalar.activation(
                        out=e, in_=tt[k].bitcast(f32), func=Act.Exp,
                        scale=1.0 / T_KD,
                    )
                    es.append(e)

                # ---- PE: mimic sum o1+o2+o3+o4 in f32r, 4 psum banks ----
                pm = ps.tile([P, 4, H], f32, tag="pm", padded_shape=[P, 4, 512])
                for jj in (0, 1):
                    for h in (0, 1):
                        for k in range(4):
                            nc.tensor.matmul(
                                pm[:, 2 * jj + h, :],
                                ident,
                                tt[k][:, jj, h * H : (h + 1) * H],
                                start=(k == 0),
                                stop=(k == 3),
                            )

                # ---- ACT: em = exp(pm/80) pair-wide, PSUM -> bf16 ----
                em = wk.tile([P, 2, C], bf16, tag="em")
                nc.scalar.activation(
                    out=em.rearrange("p j (h c) -> p (j h) c", h=2),
                    in_=pm, func=Act.Exp, scale=1.0 / (4.0 * T_KD),
                )

                # ---- DVE/Pool: muls + all per-row reductions ----
                dscrap = wk.tile([P, 2, C], bf16, tag="dscrap")

                def red(in_ap, col, op, j):
                    nc.vector.tensor_scalar(
                        out=dscrap[:, 0, :], in0=in_ap, scalar1=1.0,
                        scalar2=None, op0=Alu.mult, op1=op,
                        accum_out=sd[:, j, col : col + 1],
                    )

                prods = {}

                def mul(k, src):  # k: 0..3 teachers, 4: mimic
                    prod = wk.tile([P, 2, C], bf16, tag="prod", bufs=5,
                                   name=f"prod{k}_{g}")
                    eng = nc.gpsimd if (0 < k <= POOL_MULS) else nc.vector
                    eng.tensor_mul(out=prod, in0=src, in1=sb)
                    prods[k] = prod

                Alu_add, Alu_max = Alu.add, Alu.max
                # teacher 0: mul on DVE + its reductions
                mul(0, es[0])
                for jj, j in enumerate(blks):
                    red(es[0][:, jj, :], D_M + 0, Alu_max, j)
                    red(es[0][:, jj, :], D_A + 0, Alu_add, j)
                    red(prods[0][:, jj, :], D_B + 0, Alu_add, j)
                # S2 from the pair-wide exp(s/20) output
                for jj, j in enumerate(blks):
                    red(scr2[:, jj, :], D_S2, Alu_add, j)
                # teachers 1..3: Pool muls (1..POOL_MULS) overlap DVE reduces
                for k in (1, 2, 3):
                    mul(k, es[k])
                    for jj, j in enumerate(blks):
                        red(es[k][:, jj, :], D_M + k, Alu_max, j)
                        red(es[k][:, jj, :], D_A + k, Alu_add, j)
                for k in (1, 2, 3):
                    for jj, j in enumerate(blks):
                        red(prods[k][:, jj, :], D_B + k, Alu_add, j)
                # mimic
                mul(4, em)
                for jj, j in enumerate(blks):
                    red(em[:, jj, :], D_M + 4, Alu_max, j)
                    red(em[:, jj, :], D_A + 4, Alu_add, j)
                    red(prods[4][:, jj, :], D_B + 4, Alu_add, j)

            nc.sync.dma_start(out=st_act, in_=sa)
            nc.sync.dma_start(out=st_dve, in_=sd)

    nc.compile()
    return nc


def _get_nc():
    global _NC
    if _NC is None:
        _NC = _build()
    return _NC


def gather_stats(res):
    """Per-core stats -> full-batch [B] / [B,*] arrays."""
    sas, sds = [], []
    for r in res.results:
        sas.append(np.transpose(r["st_act"], (1, 0)).reshape(ROWS))
        sds.append(np.transpose(r["st_dve"], (1, 0, 2)).reshape(ROWS, ND))
    return np.concatenate(sas, 0), np.concatenate(sds, 0)


def kernel(outputs1, outputs2, outputs3, outputs4, out_s, targets):
    global LAST_RESULTS
    outputs1 = np.asarray(outputs1, dtype=np.float32)
    outputs2 = np.asarray(outputs2, dtype=np.float32)
    outputs3 = np.asarray(outputs3, dtype=np.float32)
    outputs4 = np.asarray(outputs4, dtype=np.float32)
    out_s = np.asarray(out_s, dtype=np.float32)
    targets = np.asarray(targets)
    nc = _get_nc()

    in_maps = []
    for k in range(NCORES):
        sl = slice(k * ROWS, (k + 1) * ROWS)
        in_maps.append(
            {
                "o1": np.ascontiguousarray(outputs1[sl]),
                "o2": np.ascontiguousarray(outputs2[sl]),
                "o3": np.ascontiguousarray(outputs3[sl]),
                "o4": np.ascontiguousarray(outputs4[sl]),
                "s": np.ascontiguousarray(out_s[sl]),
            }
        )

    def _run():
        try:
            return run_bass_kernel_spmd(
                nc, in_maps, core_ids=list(range(NCORES))
            )
        except ModuleNotFoundError:
            # BASS_TRACE set but this environment lacks the axon NTFF hook
            os.environ["BASS_NEVER_TRACE"] = "1"
            return run_bass_kernel_spmd(
                nc, in_maps, core_ids=list(range(NCORES))
            )

    res = None
    for attempt in range(3):
        try:
            res = _run()
            break
        except ModuleNotFoundError:
            raise
        except Exception:
            if attempt == 2:
                raise
            time.sleep(15 * (attempt + 1))
    LAST_RESULTS = res

    sa, sd = gather_stats(res)

    return _finalize(
        sa, sd, outputs1, outputs2, outputs3, outputs4, out_s, targets
    )


def _finalize(sa, sd, outputs1, outputs2, outputs3, outputs4, out_s, targets):
    f32 = np.float32
    tgt = np.asarray(targets).astype(np.int64)
    ar = np.arange(B)
    teacher_arrs = (outputs1, outputs2, outputs3, outputs4)

    S1 = sa.astype(np.float64)                      # sum exp(s)
    A = sd[:, D_A : D_A + 5].astype(np.float64)     # A1..A5
    Bt = sd[:, D_B : D_B + 5].astype(np.float64)    # B1..B5
    S2 = sd[:, D_S2].astype(np.float64)             # sum exp(s/20)
    # approximate row maxes, recovered from max exp
    with np.errstate(divide="ignore"):
        m_t = T_KD * np.log(sd[:, D_M : D_M + 4].astype(np.float64))   # [B,4]
        m_m = 4.0 * T_KD * np.log(sd[:, D_M + 4].astype(np.float64))   # [B] unscaled mimic

    # target-gathered logits (exact input f32 values)
    v1 = outputs1[ar, tgt]
    v2 = outputs2[ar, tgt]
    v3 = outputs3[ar, tgt]
    v4 = outputs4[ar, tgt]
    vs = out_s[ar, tgt]
    # mimic target value, exact f32 chain as in the reference
    v5 = (((v1 + v2) + v3) + v4) * f32(0.25)
    vall = np.stack([v1, v2, v3, v4, v5], 1)  # [B,5] f32

    # ---- margins: candidate rows via window test, exact host recompute ----
    margins = np.zeros((B, 5), np.float32)
    for t in range(4):
        cand = np.nonzero(vall[:, t] >= m_t[:, t] - EPS_T)[0]
        for r in cand:
            row = teacher_arrs[t][r]
            mx = row.max()
            if vall[r, t] == mx:
                margins[r, t] = mx - np.partition(row, -2)[-2]
    cand5 = np.nonzero((v1 + v2) + v3 + v4 >= m_m - EPS_M)[0]
    for r in cand5:
        row = (((outputs1[r] + outputs2[r]) + outputs3[r]) + outputs4[r]) * f32(0.25)
        mx = row.max()
        if vall[r, 4] == mx:
            margins[r, 4] = mx - np.partition(row, -2)[-2]

    # ---- exact max_preds via window + recompute ----
    mhat = m_t.max()
    rws, tch = np.nonzero(m_t >= mhat - EPS_GLOBAL)
    max_preds = np.float64(
        max(teacher_arrs[t][r].max() for r, t in zip(rws, tch))
    )

    z = margins.astype(np.float64) / T_THR
    ez = np.exp(z - z.max(1, keepdims=True))
    thr = ez / ez.sum(1, keepdims=True)

    w = vall.astype(np.float64) / max_preds
    w1 = 1.0 - ALPHA * w
    w2 = ALPHA * w

    ce = np.log(S1) - vs.astype(np.float64)                       # [B]
    kd = (T_KD * T_KD) * np.log(S2)[:, None] - T_KD * (Bt / A)    # [B,5]

    loss = w1 * ce[:, None] + w2 * kd
    per_sample = (thr * loss).sum(1)
    return np.asarray(per_sample.mean(), dtype=np.float32)
